# revision 1
# baseline (speedup 1.0000x reference)
"""Multi-head causal attention (B=4, S=2048, D=1024, H=16) on 8 trn2 cores.

Sharding: core c = (b, g) with b = c // 2 (batch), g = c % 2 (head group of 8
heads = 512 feature dims). Each core:
  - QKV projections for its 512 head-dims (weights column-sliced),
  - causal attention for its 8 heads (processed as 4 pairs of 2 heads),
  - partial output projection (Wo row-sliced); host sums the two partials
    per batch and transposes back.

On-chip layout is feature-major (transposed): x^T [D, S] resident in SBUF, so
QT = Wq^T-form and scores^T = K^T-tiles x Q^T all use natural-layout weights
as the stationary matmul operand. Softmax runs without max-subtraction
(scores ~ N(0,1) after the 1/sqrt(dh) scale, so exp is safe in fp32); the
denominator is produced by a ones-column appended to V inside the AV matmul
group. Matmuls run as float32r (full PE speed at N>=512, ~1.5e-4 rel err at
K=1024); every producer of an fp32r matmul input declares a float32r output
(walrus verifier requirement).

The walrus build here rejects instructions with more than ~1 sync-wait
command; split_waits() post-processes the scheduled BIR into raw-bass style
single-wait EventSemaphore carriers.
"""

import numpy as np

import concourse.bass as bass
import concourse.mybir as mybir
import concourse.tile as tile
from concourse.bass_utils import run_bass_kernel_spmd

B, S, D, H = 4, 2048, 1024, 16
DH = D // H              # 64
HPC = 8                  # heads per core
GD = HPC * DH            # 512 feature dims per core
PAIRS = HPC // 2         # 4 head pairs per core
P = 128
NQT = S // 512           # 4 query tiles of 512
NKT = S // P             # 16 key tiles of 128
DKT = D // P             # 8 contraction tiles for projections
OT = D // P              # 8 output row tiles
FP32 = mybir.dt.float32
FR = mybir.dt.float32r

N_CORES = 8


def _fr(ap):
    return ap.bitcast(FR)


def split_waits(nc, max_waits=1, mm_max_waits=0):
    """Split multi-wait instructions into single-wait EventSemaphore carriers
    on the same engine (this walrus codegen rejects >~1 sync wait per
    instruction; self-loading fp32/fp32r matmuls reject any on the LDW
    struct)."""
    n_carriers = 0
    for f in nc.m.functions:
        for blk in f.blocks:
            insts = blk.instructions
            i = 0
            while i < len(insts):
                inst = insts[i]
                si = inst.sync_info
                tname = type(inst).__name__
                if si is None:
                    i += 1
                    continue
                limit = mm_max_waits if tname == "InstMatmult" else max_waits
                waits = list(si.on_wait)
                if len(waits) <= limit:
                    i += 1
                    continue
                keep = waits[len(waits) - limit:] if limit else []
                extra = waits[: len(waits) - limit]
                for j, w in enumerate(extra):
                    ev = mybir.InstEventSemaphore(
                        name=nc.get_next_instruction_name(),
                        engine=inst.engine,
                        ins=[],
                        outs=[],
                        sync_info=mybir.SyncInfo(on_wait=[w], on_update=[]),
                        debug=inst.debug,
                    )
                    insts.insert(i + j, ev)
                    n_carriers += 1
                inst.sync_info = mybir.SyncInfo(
                    on_wait=keep, on_update=list(si.on_update)
                )
                i += len(extra) + 1
    return n_carriers


def build_bass(split=True):
    nc = bass.Bass()

    xt = nc.dram_tensor("xt", [D, S], FP32, kind="ExternalInput").ap()
    wq = nc.dram_tensor("wq", [D, GD], FP32, kind="ExternalInput").ap()
    wk = nc.dram_tensor("wk", [D, GD], FP32, kind="ExternalInput").ap()
    wv = nc.dram_tensor("wv", [D, GD], FP32, kind="ExternalInput").ap()
    wo = nc.dram_tensor("wo", [GD, D], FP32, kind="ExternalInput").ap()
    bq = nc.dram_tensor("bq", [P, PAIRS], FP32, kind="ExternalInput").ap()
    bk = nc.dram_tensor("bk", [P, PAIRS], FP32, kind="ExternalInput").ap()
    bv = nc.dram_tensor("bv", [1, GD], FP32, kind="ExternalInput").ap()
    bo = nc.dram_tensor("bo", [P, OT], FP32, kind="ExternalInput").ap()
    pb = nc.dram_tensor("pb", [P, NKT], FP32, kind="ExternalInput").ap()
    onesr = nc.dram_tensor("onesr", [1, P], FP32, kind="ExternalInput").ap()
    onesc = nc.dram_tensor("onesc", [P, NKT], FP32, kind="ExternalInput").ap()
    out = nc.dram_tensor("out", [D, S], FP32, kind="ExternalOutput").ap()

    xt_t = xt.rearrange("(o p) s -> p o s", p=P)
    wq_t = wq.rearrange("(o p) c -> p o c", p=P)
    wk_t = wk.rearrange("(o p) c -> p o c", p=P)
    wv_t = wv.rearrange("(o p) c -> p o c", p=P)
    wo_t = wo.rearrange("(o p) c -> p o c", p=P)
    out_t = out.rearrange("(o p) s -> p o s", p=P)

    ADD = mybir.AluOpType.add
    MUL = mybir.AluOpType.mult
    EXP = mybir.ActivationFunctionType.Exp

    with tile.TileContext(nc) as tc:
        with (
            tc.tile_pool(name="const", bufs=1) as cpool,
            tc.tile_pool(name="xp", bufs=1) as xpool,
            tc.tile_pool(name="wop", bufs=1) as wopool,
            tc.tile_pool(name="wp", bufs=2) as wpool,
            tc.tile_pool(name="qkv", bufs=2) as qkvpool,
            tc.tile_pool(name="ptp", bufs=4) as ptpool,
            tc.tile_pool(name="attp", bufs=1) as attpool,
            tc.tile_pool(name="rcp", bufs=1) as rpool,
            tc.tile_pool(name="bcp", bufs=2) as bpool,
            tc.tile_pool(name="outp", bufs=2) as outpool,
            tc.tile_pool(name="ps_proj", bufs=2, space="PSUM") as ps_proj,
            tc.tile_pool(name="ps_s", bufs=2, space="PSUM") as ps_sp,
            tc.tile_pool(name="ps_att", bufs=2, space="PSUM") as ps_attp,
            tc.tile_pool(name="ps_bc", bufs=2, space="PSUM") as ps_bcp,
        ):
            # ---- constants / global loads ----
            # fp32r ones (stationary operand of the V-bias matmul) —
            # DMA'd: memset cannot declare a float32r output
            ones_fr = cpool.tile([1, P], FP32, tag="ones_fr")
            nc.sync.dma_start(_fr(ones_fr), _fr(onesr))
            # fp32 ones rows (stationary operand of the fp32 broadcast
            # matmuls; rows 0 and 32 used)
            ones_f = cpool.tile([64, 64], FP32, tag="ones_f")
            nc.any.memset(ones_f, 1.0)

            bq_t = cpool.tile([P, PAIRS], FP32, tag="bq")
            bk_t = cpool.tile([P, PAIRS], FP32, tag="bk")
            bv_t = cpool.tile([1, GD], FP32, tag="bv")
            bo_t = cpool.tile([P, OT], FP32, tag="bo")
            pb_t = cpool.tile([P, NKT], FP32, tag="pb")
            nc.sync.dma_start(bq_t, bq)
            nc.sync.dma_start(bk_t, bk)
            nc.sync.dma_start(_fr(bv_t), _fr(bv))
            nc.sync.dma_start(bo_t, bo)
            nc.sync.dma_start(pb_t, pb)

            xt_sb = xpool.tile([P, DKT, S], FP32, tag="xt")
            for o in range(DKT):
                nc.sync.dma_start(
                    _fr(xt_sb[:, o : o + 1, :]), _fr(xt_t[:, o : o + 1, :])
                )

            wo_sb = wopool.tile([P, PAIRS, D], FP32, tag="wo")
            for o in range(PAIRS):
                nc.sync.dma_start(
                    _fr(wo_sb[:, o : o + 1, :]), _fr(wo_t[:, o : o + 1, :])
                )

            att_tiles = {}

            for pair in range(PAIRS):
                c0, c1 = pair * P, (pair + 1) * P

                wq_sb = wpool.tile([P, DKT, P], FP32, tag="wq")
                wk_sb = wpool.tile([P, DKT, P], FP32, tag="wk")
                wv_sb = wpool.tile([P, DKT, P], FP32, tag="wv")
                nc.sync.dma_start(_fr(wq_sb), _fr(wq_t[:, :, c0:c1]))
                nc.sync.dma_start(_fr(wk_sb), _fr(wk_t[:, :, c0:c1]))
                nc.sync.dma_start(_fr(wv_sb), _fr(wv_t[:, :, c0:c1]))

                qt_sb = qkvpool.tile([P, S], FP32, tag="qt")
                kt_sb = qkvpool.tile([P, S], FP32, tag="kt")
                # V in natural [s, dv] layout: [V0 | ones | V1 | ones]
                # so each head's lhsT slice [h*65 : h*65+65] carries its own
                # denominator ones-column at M index 64.
                v_sb = qkvpool.tile([P, NKT, 130], FP32, tag="v")
                nc.sync.dma_start(_fr(v_sb[:, :, 64:65]), _fr(onesc[:, :, None]))
                nc.sync.dma_start(_fr(v_sb[:, :, 129:130]), _fr(onesc[:, :, None]))

                # QT / KT projections: out [dq, s]
                for wt, bt, dst in ((wq_sb, bq_t, qt_sb), (wk_sb, bk_t, kt_sb)):
                    for q4 in range(NQT):
                        ps = ps_proj.tile([P, 512], FP32, tag="proj")
                        for k8 in range(DKT):
                            nc.tensor.matmul(
                                ps,
                                _fr(wt[:, k8, :]),
                                _fr(xt_sb[:, k8, q4 * 512 : (q4 + 1) * 512]),
                                start=(k8 == 0),
                                stop=(k8 == DKT - 1),
                            )
                        nc.vector.tensor_tensor(
                            _fr(dst[:, q4 * 512 : (q4 + 1) * 512]),
                            ps,
                            bt[:, pair : pair + 1].to_broadcast([P, 512]),
                            ADD,
                        )

                # V projection: out [s, dv] natural layout + bias via ones row
                for st in range(NKT):
                    ps = ps_proj.tile([P, 512], FP32, tag="proj")
                    psv = ps[:, :P]
                    for k8 in range(DKT):
                        nc.tensor.matmul(
                            psv,
                            _fr(xt_sb[:, k8, st * P : (st + 1) * P]),
                            _fr(wv_sb[:, k8, :]),
                            start=(k8 == 0),
                            stop=False,
                        )
                    nc.tensor.matmul(
                        psv,
                        _fr(ones_fr[0:1, 0:P]),
                        _fr(bv_t[0:1, c0:c1]),
                        start=False,
                        stop=True,
                    )
                    nc.vector.tensor_copy(_fr(v_sb[:, st, 0:64]), psv[:, 0:64])
                    nc.vector.tensor_copy(_fr(v_sb[:, st, 65:129]), psv[:, 64:128])

                # ---- attention for this head pair ----
                for Q in range(NQT):
                    q0 = Q * 512
                    n_kt = 4 * (Q + 1)
                    # per-head accumulators (fp32r matmul dst must start at
                    # partition 0): rows 0-63 = attE, row 64 = denominator
                    ps_att = [
                        ps_attp.tile([65, 512], FP32, tag="att", name="ps_att0"),
                        ps_attp.tile([65, 512], FP32, tag="att", name="ps_att1"),
                    ]

                    for kt in range(n_kt):
                        k0 = kt * P
                        diag = kt >= 4 * Q

                        pts = []
                        for h in (0, 1):
                            hp0, hp1 = h * 64, h * 64 + 64
                            ps_s = ps_sp.tile([P, 512], FP32, tag="s")
                            nc.tensor.matmul(
                                ps_s,
                                _fr(kt_sb[hp0:hp1, k0 : k0 + P]),
                                _fr(qt_sb[hp0:hp1, q0 : q0 + 512]),
                                start=True,
                                stop=True,
                            )
                            pt = ptpool.tile([P, 512], FP32, tag="pt")
                            # exp(s/8 + pad_bias)
                            nc.scalar.activation(
                                _fr(pt),
                                ps_s,
                                EXP,
                                bias=pb_t[:, kt : kt + 1],
                                scale=1.0 / 8.0,
                            )
                            if diag:
                                # zero the k > q region (keep iff q - k >= 0)
                                nc.gpsimd.affine_select(
                                    out=_fr(pt),
                                    in_=_fr(pt),
                                    compare_op=mybir.AluOpType.is_ge,
                                    fill=0.0,
                                    base=q0 - k0,
                                    channel_multiplier=-1,
                                    pattern=[[1, 512]],
                                )
                            pts.append(pt)

                        st_f = kt == 0
                        sp_f = kt == n_kt - 1
                        for h in (0, 1):
                            nc.tensor.matmul(
                                ps_att[h],
                                _fr(v_sb[:, kt, h * 65 : h * 65 + 65]),
                                _fr(pts[h]),
                                start=st_f,
                                stop=sp_f,
                            )

                    # normalize: att = attE * (1/den); denominators live at
                    # row 64 of each head bank. Reciprocal rows are placed at
                    # partitions 0 (h0) and 32 (h1) so the two fp32
                    # outer-product broadcasts use different PE row groups
                    # (concurrent) while writing separate banks at base 0.
                    recip = rpool.tile([64, 512], FP32, tag="recip")
                    nc.vector.reciprocal(recip[0:1, :], ps_att[0][64:65, :])
                    nc.vector.reciprocal(recip[32:33, :], ps_att[1][64:65, :])
                    ps_bc = [
                        ps_bcp.tile([64, 512], FP32, tag="bc", name="ps_bc0"),
                        ps_bcp.tile([64, 512], FP32, tag="bc", name="ps_bc1"),
                    ]
                    nc.tensor.matmul(
                        ps_bc[0],
                        ones_f[0:1, 0:64],
                        recip[0:1, :],
                        start=True,
                        stop=True,
                        tile_position=(0, 0),
                    )
                    nc.tensor.matmul(
                        ps_bc[1],
                        ones_f[32:33, 0:64],
                        recip[32:33, :],
                        start=True,
                        stop=True,
                        tile_position=(32, 0),
                    )
                    sb_b = bpool.tile([P, 512], FP32, tag="bc")
                    nc.vector.tensor_copy(sb_b[0:64, :], ps_bc[0])
                    nc.vector.tensor_copy(sb_b[64:128, :], ps_bc[1])
                    at = attpool.tile([P, 512], FP32, tag=f"att_{pair}_{Q}")
                    nc.vector.tensor_tensor(
                        _fr(at[0:64, :]), ps_att[0][0:64, :], sb_b[0:64, :], MUL
                    )
                    nc.vector.tensor_tensor(
                        _fr(at[64:128, :]), ps_att[1][0:64, :], sb_b[64:128, :], MUL
                    )
                    att_tiles[(pair, Q)] = at

            # ---- output projection: out^T [o, s] partial ----
            for ot in range(OT):
                for Q in range(NQT):
                    ps = ps_proj.tile([P, 512], FP32, tag="proj")
                    for pair in range(PAIRS):
                        nc.tensor.matmul(
                            ps,
                            _fr(wo_sb[:, pair, ot * P : (ot + 1) * P]),
                            _fr(att_tiles[(pair, Q)]),
                            start=(pair == 0),
                            stop=(pair == PAIRS - 1),
                        )
                    ob = outpool.tile([P, 512], FP32, tag="out")
                    nc.vector.tensor_tensor(
                        ob,
                        ps,
                        bo_t[:, ot : ot + 1].to_broadcast([P, 512]),
                        ADD,
                    )
                    nc.sync.dma_start(
                        out_t[:, ot, Q * 512 : (Q + 1) * 512], ob
                    )

    if split:
        split_waits(nc)
    return nc


_NC_CACHE = None


def _get_nc():
    global _NC_CACHE
    if _NC_CACHE is None:
        _NC_CACHE = build_bass()
    return _NC_CACHE


def make_in_maps(x, Wq, bq, Wk, bk, Wv, bv, Wo, bo, padding_mask):
    x = np.asarray(x, dtype=np.float32)
    Wq = np.asarray(Wq, dtype=np.float32)
    Wk = np.asarray(Wk, dtype=np.float32)
    Wv = np.asarray(Wv, dtype=np.float32)
    Wo = np.asarray(Wo, dtype=np.float32)
    bq = np.asarray(bq, dtype=np.float32)
    bk = np.asarray(bk, dtype=np.float32)
    bv = np.asarray(bv, dtype=np.float32)
    bo = np.asarray(bo, dtype=np.float32)
    pm = np.asarray(padding_mask)

    in_maps = []
    for c in range(N_CORES):
        b, g = c // 2, c % 2
        s0, s1 = g * GD, (g + 1) * GD
        pbias = np.where(pm[b] != 0, 0.0, -1e30).astype(np.float32)
        in_maps.append(
            {
                "xt": np.ascontiguousarray(x[b].T),
                "wq": np.ascontiguousarray(Wq[:, s0:s1]),
                "wk": np.ascontiguousarray(Wk[:, s0:s1]),
                "wv": np.ascontiguousarray(Wv[:, s0:s1]),
                "wo": np.ascontiguousarray(Wo[s0:s1, :]),
                "bq": np.ascontiguousarray(bq[s0:s1].reshape(PAIRS, P).T),
                "bk": np.ascontiguousarray(bk[s0:s1].reshape(PAIRS, P).T),
                "bv": bv[s0:s1].reshape(1, GD).copy(),
                "bo": np.ascontiguousarray(
                    (bo if g == 0 else np.zeros_like(bo)).reshape(OT, P).T
                ),
                "pb": np.ascontiguousarray(pbias.reshape(NKT, P).T),
                "onesr": np.ones((1, P), np.float32),
                "onesc": np.ones((P, NKT), np.float32),
            }
        )
    return in_maps


def assemble(results):
    out = np.empty((B, S, D), dtype=np.float32)
    for b in range(B):
        acc = results[2 * b]["out"] + results[2 * b + 1]["out"]
        out[b] = acc.T
    return out


def kernel(**inputs):
    nc = _get_nc()
    in_maps = make_in_maps(**inputs)
    res = run_bass_kernel_spmd(nc, in_maps, list(range(N_CORES)))
    return assemble(res.results)


if __name__ == "__main__":
    rng = np.random.default_rng(0)
    inputs = {
        "x": rng.standard_normal((B, S, D), dtype=np.float32),
        "Wq": rng.standard_normal((D, D), dtype=np.float32) / 32,
        "bq": np.zeros(D, np.float32),
        "Wk": rng.standard_normal((D, D), dtype=np.float32) / 32,
        "bk": np.zeros(D, np.float32),
        "Wv": rng.standard_normal((D, D), dtype=np.float32) / 32,
        "bv": np.zeros(D, np.float32),
        "Wo": rng.standard_normal((D, D), dtype=np.float32) / 32,
        "bo": np.zeros(D, np.float32),
        "padding_mask": np.ones((B, S), np.int32),
    }
    out = kernel(**inputs)
    print(out.shape, out.dtype)

